# revision 33
# baseline (speedup 1.0000x reference)
"""Multi-head self-attention (B=4, S=2048, hidden=1024, 16 heads, d_k=64,
causal) on 8 Trainium2 NeuronCores.

Sharding: core c handles batch b = c//2 and head-group hg = c%2 (8 heads =
512 hidden dims). Each core computes Q/K/V for its heads, causal attention,
and a partial output projection against its wo column-slice; the host sums
the two partials per batch and adds bo.

Performance notes (vs the fp32r baseline):
  * everything is bf16 (halves DMA + SBUF traffic; PE rate identical).
  * score matmuls contract over d_k=64 only; heads 2j / 2j+1 live at
    partitions 0-63 / 64-127 so their score MMs are issued back-to-back and
    run CONCURRENTLY in different PE row-groups (row tiling).
  * QKV token-chunks are interleaved into the attention loop so the PE
    never idles long enough for the HAM clock gate to re-throttle it to
    1.2 GHz (the baseline ran its whole attention phase cold).
  * exp() runs on the scalar engine over merged [128, 2*512] chunks
    (amortizes the ~352-cycle ACTIVATE pipeline fill).

Device layouts (SBUF is [128 partitions, free]):
  x^T   [in=8*128, tok]      host-transposed activations
  Q^T/K^T [dout=4*128, tok]  head h occupies rows (h%2)*64..+64 of dblk h//2
  V     [tok, head, d_k+1]   65th column is ones so the PV matmul also
                             produces the softmax denominator row
  scores are computed transposed: S^T[k, q] = K @ Q^T, so softmax's sum
  over keys becomes a matmul contraction instead of a partition reduction.

PSUM (8 banks): 2 for QKV/O-proj accumulation chains, 4 as the score ring
(two double-buffered [128, 2, 512] exp chunks), 2 for the PV accumulators
of the head pair in flight.
"""

import os
import sys

for _p in (
    "/root/.axon_site",
    "/root/.axon_site/_ro/trn_rl_repo",
    "/root/.axon_site/_ro/pypackages",
    "/opt/trn_rl_repo",
):
    if os.path.isdir(_p) and _p not in sys.path:
        sys.path.append(_p)

import numpy as np
import ml_dtypes

import concourse.mybir as mybir
import concourse.tile as tile
from concourse import bacc
from concourse.bass import ts
from concourse.bass_utils import run_bass_kernel_spmd

F32 = mybir.dt.float32
BF16 = mybir.dt.bfloat16
AF = mybir.ActivationFunctionType
ALU = mybir.AluOpType

B, S, HID = 4, 2048, 1024
HEADS, DK = 16, 64
NCORES = 8
HPC = HEADS // 2          # 8 heads per core
HSL = HPC * DK            # 512-dim hidden slice per core
TC = 512                  # token/query chunk
NTC = S // TC             # 4
NTB = S // 128            # 16 token blocks
NEG = -1.0e30


def build_nc(debug_dumps=False):
    nc = bacc.Bacc("TRN2", target_bir_lowering=False, debug=False)

    xT = nc.dram_tensor("xT", [HID, S], BF16, kind="ExternalInput").ap()
    wqT = nc.dram_tensor("wqT", [HID, HSL], BF16, kind="ExternalInput").ap()
    wkT = nc.dram_tensor("wkT", [HID, HSL], BF16, kind="ExternalInput").ap()
    wvT = nc.dram_tensor("wvT", [HID, HSL], BF16, kind="ExternalInput").ap()
    woT = nc.dram_tensor("woT", [HSL, HID], BF16, kind="ExternalInput").ap()
    bq = nc.dram_tensor("bq", [HSL], F32, kind="ExternalInput").ap()
    bk = nc.dram_tensor("bk", [HSL], F32, kind="ExternalInput").ap()
    bv_rep = nc.dram_tensor("bv_rep", [128, HSL], F32, kind="ExternalInput").ap()
    dmask = nc.dram_tensor("dmask", [128, 128], F32, kind="ExternalInput").ap()
    out = nc.dram_tensor("out_p", [S, HID], BF16, kind="ExternalOutput").ap()
    if debug_dumps:
        qT_d = nc.dram_tensor("qT_d", [HSL, S], F32, kind="ExternalOutput").ap()
        kT_d = nc.dram_tensor("kT_d", [HSL, S], F32, kind="ExternalOutput").ap()
        v_d = nc.dram_tensor(
            "v_d", [128, NTB * HPC * (DK + 1)], F32, kind="ExternalOutput"
        ).ap()
        aT_d = nc.dram_tensor("aT_d", [HSL, S], F32, kind="ExternalOutput").ap()

    xT_r = xT.rearrange("(ic p) t -> p ic t", p=128)      # [128, 8, S]
    wqT_r = wqT.rearrange("(ic p) o -> p ic o", p=128)    # [128, 8, HSL]
    wkT_r = wkT.rearrange("(ic p) o -> p ic o", p=128)
    wvT_r = wvT.rearrange("(ic p) o -> p ic o", p=128)
    woT_r = woT.rearrange("(hb p) o -> p hb o", p=128)    # [128, 4, HID]
    bq_r = bq.rearrange("(d p) -> p d", p=128)            # [128, 4]
    bk_r = bk.rearrange("(d p) -> p d", p=128)
    out_r = out.rearrange("(tb p) o -> p tb o", p=128)    # [128, 16, HID]

    with tile.TileContext(nc) as tc:
        with (
            tc.tile_pool(name="mn", bufs=1) as mn,
            tc.tile_pool(name="xt", bufs=2) as xt_pool,
            tc.tile_pool(name="pt", bufs=4) as pt_pool,
            tc.tile_pool(name="sm", bufs=3) as sm_pool,
            tc.tile_pool(name="ot", bufs=2) as ot_pool,
            tc.tile_pool(name="psq", bufs=2, space="PSUM") as psq,
            tc.tile_pool(name="ring", bufs=2, space="PSUM") as ringp,
            tc.tile_pool(name="pso", bufs=2, space="PSUM") as psop,
        ):
            # ---- persistent SBUF tensors ----
            qT_sb = mn.tile([128, 4, S], BF16, tag="qT")
            kT_sb = mn.tile([128, 4, S], BF16, tag="kT")
            v_sb = mn.tile([128, NTB, HPC, DK + 1], BF16, tag="v")
            aT_sb = mn.tile([128, 4, S], BF16, tag="aT")
            wq_sb = mn.tile([128, 8, HSL], BF16, tag="wq")
            wk_sb = mn.tile([128, 8, HSL], BF16, tag="wk")
            wv_sb = mn.tile([128, 8, HSL], BF16, tag="wv")
            wo_sb = mn.tile([128, 4, HID], BF16, tag="wo")
            bq_sb = mn.tile([128, 4], F32, tag="bq")
            bk_sb = mn.tile([128, 4], F32, tag="bk")
            bv_sb = mn.tile([128, HSL], F32, tag="bv")
            dm_sb = mn.tile([128, 128], F32, tag="dm")

            nc.vector.memset(v_sb[:, :, :, DK], 1.0)

            # prologue DMAs: the first Q chain accumulates over ic=0..7 in
            # order, so stream x0/wq in 2-ic pieces — the PE starts after
            # the first ~0.5MB instead of the full 2MB; wo is not needed
            # until the output projection and goes last.
            xt0 = xt_pool.tile([128, 8, TC], BF16, tag="xt", name="xt0")
            nc.sync.dma_start(xt0[:, 0:2, :], xT_r[:, 0:2, ts(0, TC)])
            nc.sync.dma_start(wq_sb[:, 0:2, :], wqT_r[:, 0:2, :])
            nc.sync.dma_start(bq_sb[:], bq_r)
            for i2 in range(2, 8, 2):
                nc.sync.dma_start(
                    xt0[:, i2 : i2 + 2, :], xT_r[:, i2 : i2 + 2, ts(0, TC)]
                )
                nc.sync.dma_start(
                    wq_sb[:, i2 : i2 + 2, :], wqT_r[:, i2 : i2 + 2, :]
                )
            nc.sync.dma_start(wk_sb[:], wkT_r)
            nc.sync.dma_start(bk_sb[:], bk_r)
            nc.sync.dma_start(wv_sb[:], wvT_r)
            nc.sync.dma_start(bv_sb[:], bv_rep)
            nc.sync.dma_start(dm_sb[:], dmask)
            nc.sync.dma_start(wo_sb[:], woT_r)

            xts = {0: xt0}

            def qkv_chain(tci, unit):
                """One QKV projection chain (8 accumulating MMs + bias add).
                unit 0-3: Q dblk; 4-7: K dblk; 8-11: V tbl."""
                if tci not in xts:
                    xt = xt_pool.tile([128, 8, TC], BF16, tag="xt",
                                      name=f"xt{tci}")
                    nc.sync.dma_start(xt[:], xT_r[:, :, ts(tci, TC)])
                    xts[tci] = xt
                xt = xts[tci]
                if unit < 8:
                    w_sb, b_sb, dst = (
                        (wq_sb, bq_sb, qT_sb) if unit < 4
                        else (wk_sb, bk_sb, kT_sb)
                    )
                    dblk = unit % 4
                    ps = psq.tile([128, TC], F32, tag="ps")
                    for ic in range(8):
                        nc.tensor.matmul(
                            ps[:],
                            w_sb[:, ic, ts(dblk, 128)],
                            xt[:, ic, :],
                            start=(ic == 0),
                            stop=(ic == 7),
                        )
                    nc.vector.tensor_tensor(
                        dst[:, dblk, ts(tci, TC)],
                        ps[:],
                        b_sb[:, dblk : dblk + 1].to_broadcast((128, TC)),
                        ALU.add,
                    )
                else:
                    tbl = unit - 8
                    ps = psq.tile([128, TC], F32, tag="ps")
                    for ic in range(8):
                        nc.tensor.matmul(
                            ps[:],
                            xt[:, ic, ts(tbl, 128)],
                            wv_sb[:, ic, :],
                            start=(ic == 0),
                            stop=(ic == 7),
                        )
                    tb = tci * 4 + tbl
                    nc.vector.tensor_tensor(
                        v_sb[:, tb, :, 0:DK],
                        ps.rearrange("p (h d) -> p h d", d=DK),
                        bv_sb.rearrange("p (h d) -> p h d", d=DK),
                        ALU.add,
                    )

            def emit_qkv(tci):
                for unit in range(12):
                    qkv_chain(tci, unit)

            ots = {}
            tail_mode = [False]

            def oproj_half(tb, half):
                """Half of the output projection for one 128-token block."""
                if half == 0:
                    ots[tb] = ot_pool.tile([128, HID], BF16, tag="ot",
                                           name=f"ot{tb}")
                ot = ots[tb]
                ps = psq.tile([128, 512], F32, tag="ps")
                for hb in range(4):
                    nc.tensor.matmul(
                        ps[:],
                        aT_sb[:, hb, ts(tb, 128)],
                        wo_sb[:, hb, ts(half, 512)],
                        start=(hb == 0),
                        stop=(hb == 3),
                    )
                if tail_mode[0]:
                    # in the tail the vector queue is serialized behind the
                    # final normalize; the scalar engine is idle after the
                    # last exp and can drain PSUM (same path as exp)
                    nc.scalar.activation(ot[:, ts(half, 512)], ps[:], AF.Copy)
                else:
                    nc.vector.tensor_copy(ot[:, ts(half, 512)], ps[:])
                nc.sync.dma_start(
                    out_r[:, tb, ts(half, 512)], ot[:, ts(half, 512)]
                )
                if half == 1:
                    del ots[tb]

            # ---- paced filler: PE work (QKV chains / O-proj halves) fed
            # into the attention loop's exp-wait gaps. est[0] tracks emitted
            # attention-PE-µs, est[1] scalar-µs; when the PE falls behind
            # the scalar engine, pop filler units to keep HAM at K=8/8.
            # Each unit carries a deadline: the attention chunk that consumes
            # its output; drain() force-emits before that chunk starts, so
            # emission order always respects data flow.
            filler = []          # list of (cost_us, fn, deadline_qc)
            est = [0.0, 0.0]     # [pe_us, scalar_us]

            def fill():
                # at most one unit per call: spreads filler uniformly so it
                # plugs PE gaps without forming long bursts that delay the
                # score matmuls feeding the scalar engine. The last 8 units
                # are reserved to bridge the final pair's exp+normalize
                # latency so the PE stays warm into the last O-proj blocks.
                if len(filler) > 8 and est[0] < est[1] + 3.0:
                    cost, fn, _ = filler.pop(0)
                    fn()
                    est[0] += cost

            def drain(pidx):
                while filler and filler[0][2] <= pidx:
                    cost, fn, _ = filler.pop(0)
                    fn()
                    est[0] += cost

            class AttPair:
                """Causal attention for head pair (2*pr, 2*pr+1), query chunk
                qc. Scores for the two heads issue adjacently into different
                PE row groups (partitions 0-63 vs 64-127) and overlap. PV
                runs one kb behind exp so the PE always has exp-independent
                work in flight. Emission is split prologue/body/tail so the
                next pair's first score chunks issue before this pair's tail
                — the scalar engine never waits at a pair boundary."""

                def __init__(self, qc, pr):
                    self.qc, self.pr = qc, pr
                    self.nkb = 4 * qc + 4
                    self.q0 = qc * TC
                    self.psos = [
                        psop.tile([DK + 1, TC], F32, tag="ops", name=f"ops{j}")
                        for j in range(2)
                    ]
                    self.pts = [None] * self.nkb
                    self.chunks = [None] * self.nkb
                    self.emit_scores(0)
                    self.emit_scores(1)

                def emit_scores(self, kb):
                    cs = max(0, kb * 128 - self.q0)
                    # [128, 2, 512] = 2 PSUM banks; pool-rotated (bufs=2) so
                    # the next chunk's scores overlap this chunk's exp
                    ch = ringp.tile([128, 2, TC], F32, tag="spc", name="spc")
                    self.chunks[kb] = (ch, cs)
                    for j, off in enumerate((0, DK)):
                        nc.tensor.matmul(
                            ch[:, j, cs:TC],
                            kT_sb[off : off + DK, self.pr, ts(kb, 128)],
                            qT_sb[off : off + DK, self.pr,
                                  self.q0 + cs : self.q0 + TC],
                            start=True,
                            stop=True,
                        )
                    if kb >= 4 * self.qc:  # diagonal: causal mask
                        for j in range(2):
                            nc.vector.tensor_tensor(
                                ch[:, j, cs : cs + 128],
                                ch[:, j, cs : cs + 128],
                                dm_sb[:],
                                ALU.add,
                            )
                    est[0] += (TC - cs) / 2400 + 0.05

                def emit_exp(self, kb):
                    ch, cs = self.chunks[kb]
                    pt = pt_pool.tile([128, 2, TC], BF16, tag="pt", name="pt")
                    # one strided ACTIVATE covers both heads' slots, trimmed
                    # to the causally-live columns
                    nc.scalar.activation(
                        pt[:, :, cs:TC], ch[:, :, cs:TC], AF.Exp, scale=0.125
                    )
                    self.pts[kb] = pt
                    est[1] += (2 * (TC - cs) + 352) / 1200

                def emit_pv(self, kb):
                    cs = self.chunks[kb][1]
                    pt = self.pts[kb]
                    for j in range(2):
                        nc.tensor.matmul(
                            self.psos[j][:, cs:TC],
                            v_sb[:, kb, 2 * self.pr + j, :],
                            pt[:, j, cs:TC],
                            start=(kb == 0),
                            stop=(kb == self.nkb - 1),
                        )
                    est[0] += 2 * (TC - cs) / 2400 + 0.05

                def body(self):
                    for kb in range(self.nkb):
                        self.emit_exp(kb)
                        if kb > 0:
                            self.emit_pv(kb - 1)
                        if kb + 2 < self.nkb:
                            self.emit_scores(kb + 2)
                        fill()

                def tail(self):
                    self.emit_pv(self.nkb - 1)
                    for j in range(2):
                        ops = self.psos[j]
                        rc = sm_pool.tile([1, TC], F32, tag="rc")
                        # custom-DVE ops mishandle partition-offset inputs:
                        # stage the sums row at partition 0
                        lsb = sm_pool.tile([1, TC], F32, tag="lsb")
                        nc.vector.tensor_copy(lsb[:], ops[DK : DK + 1, :])
                        nc.vector.reciprocal_approx_fast(rc[:], lsb[:])
                        bcs = sm_pool.tile([DK, TC], F32, tag="bcs")
                        nc.gpsimd.partition_broadcast(bcs[:], rc[:])
                        if j == 0:
                            nc.vector.tensor_tensor(
                                aT_sb[0:DK, self.pr, ts(self.qc, TC)],
                                ops[0:DK, :],
                                bcs[:],
                                ALU.mult,
                            )
                        else:
                            tmp = sm_pool.tile([DK, TC], BF16, tag="tmp")
                            nc.vector.tensor_tensor(
                                tmp[:], ops[0:DK, :], bcs[:], ALU.mult
                            )
                            # engines are lane-locked; DMA shifts partitions
                            nc.sync.dma_start(
                                aT_sb[DK:128, self.pr, ts(self.qc, TC)],
                                tmp[:],
                            )

            # ---- emission schedule ----
            # Minimal eager prologue: Q/K for dblk 0 and the four V chains
            # of token chunk 0 — exactly what attention (qc=0, pair 0)
            # consumes — so the scalar engine starts exp() as early as
            # possible. Everything else (remaining QKV chains, the output
            # projection) drips into the attention loop as paced filler so
            # the PE never idles long enough for HAM to re-throttle it to
            # 1.2 GHz. Deadlines are pair indices (qc*4+pr): drain() force-
            # emits a unit before the attention pair that consumes it.
            qkvf = lambda t, u, dl: (1.8, lambda: qkv_chain(t, u), dl)
            oprf = lambda t, h: (1.0, lambda: oproj_half(t, h), 99)
            emit_qkv(0)
            filler += [qkvf(1, u, 4) for u in range(12)]
            prev = None
            for qc in range(NTC):
                if qc == 0:
                    filler += [qkvf(2, u, 8) for u in range(12)]
                elif qc == 1:
                    filler += [qkvf(3, u, 12) for u in range(12)]
                elif qc == 2:
                    filler += [oprf(t, h) for t in range(0, 4)
                               for h in range(2)]
                elif qc == 3:
                    filler += [oprf(t, h) for t in range(4, 12)
                               for h in range(2)]
                for pr in range(4):
                    drain(qc * 4 + pr)
                    st = AttPair(qc, pr)   # emits scores(0),(1)
                    if prev is not None:
                        prev.tail()
                    st.body()
                    prev = st
            # flush leftover filler BEFORE the last pair's tail: the PE
            # chews through it while the final exps/normalize drain
            tail_mode[0] = True
            for _, fn, _d in filler:
                fn()
            prev.tail()
            for tb in range(12, 16):
                for half in range(2):
                    oproj_half(tb, half)

            if debug_dumps:
                nc.sync.dma_start(
                    qT_d.rearrange("(d p) t -> p d t", p=128), qT_sb[:]
                )
                nc.sync.dma_start(
                    kT_d.rearrange("(d p) t -> p d t", p=128), kT_sb[:]
                )
                nc.sync.dma_start(
                    v_d[:], v_sb.rearrange("p a b c -> p (a b c)")
                )
                nc.sync.dma_start(
                    aT_d.rearrange("(d p) t -> p d t", p=128), aT_sb[:]
                )
    nc.compile()
    return nc


_NC = None


def _get_nc():
    global _NC
    if _NC is None:
        _NC = build_nc()
    return _NC


def _numpy_reference(x, attn_mask, wq, bq, wk, bk, wv, bv, wo, bo):
    """Fallback for a non-causal mask (never hit with the standard inputs)."""
    Bsz, Seq, D = x.shape
    scale = 1.0 / np.sqrt(DK)

    def proj(w, b):
        y = x @ w.T + b
        return y.reshape(Bsz, Seq, HEADS, DK).transpose(0, 2, 1, 3)

    q, k, v = proj(wq, bq), proj(wk, bk), proj(wv, bv)
    scores = np.einsum("bhqd,bhkd->bhqk", q, k) * scale
    scores = np.where(attn_mask == 0, np.float32(-1e9), scores)
    scores = scores - scores.max(axis=-1, keepdims=True)
    p = np.exp(scores)
    p /= p.sum(axis=-1, keepdims=True)
    o = np.einsum("bhqk,bhkd->bhqd", p, v)
    o = o.transpose(0, 2, 1, 3).reshape(Bsz, Seq, D)
    return o @ wo.T + bo


def kernel(x, attn_mask, wq, bq, wk, bk, wv, bv, wo, bo, **_unused):
    x = np.asarray(x, np.float32)
    attn_mask = np.asarray(attn_mask)
    wq, bq = np.asarray(wq, np.float32), np.asarray(bq, np.float32)
    wk, bk = np.asarray(wk, np.float32), np.asarray(bk, np.float32)
    wv, bv = np.asarray(wv, np.float32), np.asarray(bv, np.float32)
    wo, bo = np.asarray(wo, np.float32), np.asarray(bo, np.float32)

    causal = np.array_equal(
        np.asarray(attn_mask).reshape(S, S) != 0, np.tril(np.ones((S, S), bool))
    )
    if not causal:
        return _numpy_reference(x, attn_mask, wq, bq, wk, bk, wv, bv, wo, bo)

    def b16(a):
        return np.ascontiguousarray(a, np.float32).astype(ml_dtypes.bfloat16)

    tri = np.where(
        np.arange(128)[:, None] <= np.arange(128)[None, :], 0.0, NEG
    ).astype(np.float32)

    in_maps = []
    for c in range(NCORES):
        b, hg = c // 2, c % 2
        sl = slice(hg * HSL, (hg + 1) * HSL)
        in_maps.append(
            {
                "xT": b16(x[b].T),
                "wqT": b16(wq[sl, :].T),
                "wkT": b16(wk[sl, :].T),
                "wvT": b16(wv[sl, :].T),
                "woT": b16(wo[:, sl].T),
                "bq": np.ascontiguousarray(bq[sl]),
                "bk": np.ascontiguousarray(bk[sl]),
                "bv_rep": np.tile(bv[sl][None, :], (128, 1)),
                "dmask": tri,
            }
        )

    res = run_bass_kernel_spmd(
        _get_nc(), in_maps, core_ids=list(range(NCORES)), **_RUN_KWARGS
    )
    if _RUN_RESULTS is not None:
        _RUN_RESULTS.append(res)

    out = np.empty((B, S, HID), np.float32)
    for b in range(B):
        out[b] = (
            res.results[2 * b]["out_p"].astype(np.float32)
            + res.results[2 * b + 1]["out_p"].astype(np.float32)
            + bo
        )
    return out


# test.py can set these to enable tracing / inspect profile results.
_RUN_KWARGS = {}
_RUN_RESULTS = None


# revision 34
# speedup vs baseline: 1.0134x; 1.0134x over previous
"""Multi-head self-attention (B=4, S=2048, hidden=1024, 16 heads, d_k=64,
causal) on 8 Trainium2 NeuronCores.

Sharding: core c handles batch b = c//2 and head-group hg = c%2 (8 heads =
512 hidden dims). Each core computes Q/K/V for its heads, causal attention,
and a partial output projection against its wo column-slice; the host sums
the two partials per batch and adds bo.

Performance notes (vs the fp32r baseline):
  * everything is bf16 (halves DMA + SBUF traffic; PE rate identical).
  * score matmuls contract over d_k=64 only; heads 2j / 2j+1 live at
    partitions 0-63 / 64-127 so their score MMs are issued back-to-back and
    run CONCURRENTLY in different PE row-groups (row tiling).
  * QKV token-chunks are interleaved into the attention loop so the PE
    never idles long enough for the HAM clock gate to re-throttle it to
    1.2 GHz (the baseline ran its whole attention phase cold).
  * exp() runs on the scalar engine over merged [128, 2*512] chunks
    (amortizes the ~352-cycle ACTIVATE pipeline fill).

Device layouts (SBUF is [128 partitions, free]):
  x^T   [in=8*128, tok]      host-transposed activations
  Q^T/K^T [dout=4*128, tok]  head h occupies rows (h%2)*64..+64 of dblk h//2
  V     [tok, head, d_k+1]   65th column is ones so the PV matmul also
                             produces the softmax denominator row
  scores are computed transposed: S^T[k, q] = K @ Q^T, so softmax's sum
  over keys becomes a matmul contraction instead of a partition reduction.

PSUM (8 banks): 2 for QKV/O-proj accumulation chains, 4 as the score ring
(two double-buffered [128, 2, 512] exp chunks), 2 for the PV accumulators
of the head pair in flight.
"""

import os
import sys

for _p in (
    "/root/.axon_site",
    "/root/.axon_site/_ro/trn_rl_repo",
    "/root/.axon_site/_ro/pypackages",
    "/opt/trn_rl_repo",
):
    if os.path.isdir(_p) and _p not in sys.path:
        sys.path.append(_p)

import numpy as np
import ml_dtypes

import concourse.mybir as mybir
import concourse.tile as tile
from concourse import bacc
from concourse.bass import ts
from concourse.bass_utils import run_bass_kernel_spmd

F32 = mybir.dt.float32
BF16 = mybir.dt.bfloat16
AF = mybir.ActivationFunctionType
ALU = mybir.AluOpType

B, S, HID = 4, 2048, 1024
HEADS, DK = 16, 64
NCORES = 8
HPC = HEADS // 2          # 8 heads per core
HSL = HPC * DK            # 512-dim hidden slice per core
TC = 512                  # token/query chunk
NTC = S // TC             # 4
NTB = S // 128            # 16 token blocks
NEG = -1.0e30


def build_nc(debug_dumps=False):
    nc = bacc.Bacc("TRN2", target_bir_lowering=False, debug=False)

    xT = nc.dram_tensor("xT", [HID, S], BF16, kind="ExternalInput").ap()
    wqT = nc.dram_tensor("wqT", [HID, HSL], BF16, kind="ExternalInput").ap()
    wkT = nc.dram_tensor("wkT", [HID, HSL], BF16, kind="ExternalInput").ap()
    wvT = nc.dram_tensor("wvT", [HID, HSL], BF16, kind="ExternalInput").ap()
    woT = nc.dram_tensor("woT", [HSL, HID], BF16, kind="ExternalInput").ap()
    bq = nc.dram_tensor("bq", [HSL], F32, kind="ExternalInput").ap()
    bk = nc.dram_tensor("bk", [HSL], F32, kind="ExternalInput").ap()
    bv_rep = nc.dram_tensor("bv_rep", [128, HSL], F32, kind="ExternalInput").ap()
    dmask = nc.dram_tensor("dmask", [128, 128], F32, kind="ExternalInput").ap()
    out = nc.dram_tensor("out_p", [S, HID], BF16, kind="ExternalOutput").ap()
    if debug_dumps:
        qT_d = nc.dram_tensor("qT_d", [HSL, S], F32, kind="ExternalOutput").ap()
        kT_d = nc.dram_tensor("kT_d", [HSL, S], F32, kind="ExternalOutput").ap()
        v_d = nc.dram_tensor(
            "v_d", [128, NTB * HPC * (DK + 1)], F32, kind="ExternalOutput"
        ).ap()
        aT_d = nc.dram_tensor("aT_d", [HSL, S], F32, kind="ExternalOutput").ap()

    xT_r = xT.rearrange("(ic p) t -> p ic t", p=128)      # [128, 8, S]
    wqT_r = wqT.rearrange("(ic p) o -> p ic o", p=128)    # [128, 8, HSL]
    wkT_r = wkT.rearrange("(ic p) o -> p ic o", p=128)
    wvT_r = wvT.rearrange("(ic p) o -> p ic o", p=128)
    woT_r = woT.rearrange("(hb p) o -> p hb o", p=128)    # [128, 4, HID]
    bq_r = bq.rearrange("(d p) -> p d", p=128)            # [128, 4]
    bk_r = bk.rearrange("(d p) -> p d", p=128)
    out_r = out.rearrange("(tb p) o -> p tb o", p=128)    # [128, 16, HID]

    with tile.TileContext(nc) as tc:
        with (
            tc.tile_pool(name="mn", bufs=1) as mn,
            tc.tile_pool(name="xt", bufs=2) as xt_pool,
            tc.tile_pool(name="pt", bufs=4) as pt_pool,
            tc.tile_pool(name="sm", bufs=3) as sm_pool,
            tc.tile_pool(name="ot", bufs=2) as ot_pool,
            tc.tile_pool(name="psq", bufs=2, space="PSUM") as psq,
            tc.tile_pool(name="ring", bufs=2, space="PSUM") as ringp,
            tc.tile_pool(name="pso", bufs=2, space="PSUM") as psop,
        ):
            # ---- persistent SBUF tensors ----
            qT_sb = mn.tile([128, 4, S], BF16, tag="qT")
            kT_sb = mn.tile([128, 4, S], BF16, tag="kT")
            v_sb = mn.tile([128, NTB, HPC, DK + 1], BF16, tag="v")
            aT_sb = mn.tile([128, 4, S], BF16, tag="aT")
            wq_sb = mn.tile([128, 8, HSL], BF16, tag="wq")
            wk_sb = mn.tile([128, 8, HSL], BF16, tag="wk")
            wv_sb = mn.tile([128, 8, HSL], BF16, tag="wv")
            wo_sb = mn.tile([128, 4, HID], BF16, tag="wo")
            bq_sb = mn.tile([128, 4], F32, tag="bq")
            bk_sb = mn.tile([128, 4], F32, tag="bk")
            bv_sb = mn.tile([128, HSL], F32, tag="bv")
            dm_sb = mn.tile([128, 128], F32, tag="dm")

            nc.vector.memset(v_sb[:, :, :, DK], 1.0)

            # prologue DMAs: the first Q chain accumulates over ic=0..7 in
            # order, so stream x0/wq in 2-ic pieces — the PE starts after
            # the first ~0.5MB instead of the full 2MB; wo is not needed
            # until the output projection and goes last.
            xt0 = xt_pool.tile([128, 8, TC], BF16, tag="xt", name="xt0")
            nc.sync.dma_start(xt0[:, 0:2, :], xT_r[:, 0:2, ts(0, TC)])
            nc.sync.dma_start(wq_sb[:, 0:2, :], wqT_r[:, 0:2, :])
            nc.sync.dma_start(bq_sb[:], bq_r)
            for i2 in range(2, 8, 2):
                nc.sync.dma_start(
                    xt0[:, i2 : i2 + 2, :], xT_r[:, i2 : i2 + 2, ts(0, TC)]
                )
                nc.sync.dma_start(
                    wq_sb[:, i2 : i2 + 2, :], wqT_r[:, i2 : i2 + 2, :]
                )
            nc.sync.dma_start(wk_sb[:], wkT_r)
            nc.sync.dma_start(bk_sb[:], bk_r)
            nc.sync.dma_start(wv_sb[:], wvT_r)
            nc.sync.dma_start(bv_sb[:], bv_rep)
            nc.sync.dma_start(dm_sb[:], dmask)
            nc.sync.dma_start(wo_sb[:], woT_r)

            xts = {0: xt0}

            def qkv_chain(tci, unit):
                """One QKV projection chain (8 accumulating MMs + bias add).
                unit 0-3: Q dblk; 4-7: K dblk; 8-11: V tbl."""
                if tci not in xts:
                    xt = xt_pool.tile([128, 8, TC], BF16, tag="xt",
                                      name=f"xt{tci}")
                    nc.sync.dma_start(xt[:], xT_r[:, :, ts(tci, TC)])
                    xts[tci] = xt
                xt = xts[tci]
                if unit < 8:
                    w_sb, b_sb, dst = (
                        (wq_sb, bq_sb, qT_sb) if unit < 4
                        else (wk_sb, bk_sb, kT_sb)
                    )
                    dblk = unit % 4
                    ps = psq.tile([128, TC], F32, tag="ps")
                    for ic in range(8):
                        nc.tensor.matmul(
                            ps[:],
                            w_sb[:, ic, ts(dblk, 128)],
                            xt[:, ic, :],
                            start=(ic == 0),
                            stop=(ic == 7),
                        )
                    nc.vector.tensor_tensor(
                        dst[:, dblk, ts(tci, TC)],
                        ps[:],
                        b_sb[:, dblk : dblk + 1].to_broadcast((128, TC)),
                        ALU.add,
                    )
                else:
                    tbl = unit - 8
                    ps = psq.tile([128, TC], F32, tag="ps")
                    for ic in range(8):
                        nc.tensor.matmul(
                            ps[:],
                            xt[:, ic, ts(tbl, 128)],
                            wv_sb[:, ic, :],
                            start=(ic == 0),
                            stop=(ic == 7),
                        )
                    tb = tci * 4 + tbl
                    nc.vector.tensor_tensor(
                        v_sb[:, tb, :, 0:DK],
                        ps.rearrange("p (h d) -> p h d", d=DK),
                        bv_sb.rearrange("p (h d) -> p h d", d=DK),
                        ALU.add,
                    )

            def emit_qkv(tci):
                for unit in range(12):
                    qkv_chain(tci, unit)

            ots = {}
            tail_mode = [False]

            def oproj_half(tb, half):
                """Half of the output projection for one 128-token block."""
                if half == 0:
                    ots[tb] = ot_pool.tile([128, HID], BF16, tag="ot",
                                           name=f"ot{tb}")
                ot = ots[tb]
                ps = psq.tile([128, 512], F32, tag="ps")
                for hb in range(4):
                    nc.tensor.matmul(
                        ps[:],
                        aT_sb[:, hb, ts(tb, 128)],
                        wo_sb[:, hb, ts(half, 512)],
                        start=(hb == 0),
                        stop=(hb == 3),
                    )
                if tail_mode[0]:
                    # in the tail the vector queue is serialized behind the
                    # final normalize; the scalar engine is idle after the
                    # last exp and can drain PSUM (same path as exp)
                    nc.scalar.activation(ot[:, ts(half, 512)], ps[:], AF.Copy)
                else:
                    nc.vector.tensor_copy(ot[:, ts(half, 512)], ps[:])
                nc.sync.dma_start(
                    out_r[:, tb, ts(half, 512)], ot[:, ts(half, 512)]
                )
                if half == 1:
                    del ots[tb]

            # ---- paced filler: PE work (QKV chains / O-proj halves) fed
            # into the attention loop's exp-wait gaps. est[0] tracks emitted
            # attention-PE-µs, est[1] scalar-µs; when the PE falls behind
            # the scalar engine, pop filler units to keep HAM at K=8/8.
            # Each unit carries a deadline: the attention chunk that consumes
            # its output; drain() force-emits before that chunk starts, so
            # emission order always respects data flow.
            filler = []          # list of (cost_us, fn, deadline_qc)
            est = [0.0, 0.0]     # [pe_us, scalar_us]

            def fill():
                # at most one unit per call: spreads filler uniformly so it
                # plugs PE gaps without forming long bursts that delay the
                # score matmuls feeding the scalar engine. The last 8 units
                # are reserved to bridge the final pair's exp+normalize
                # latency so the PE stays warm into the last O-proj blocks.
                if len(filler) > 8 and est[0] < est[1] + 3.0:
                    cost, fn, _ = filler.pop(0)
                    fn()
                    est[0] += cost

            def drain(pidx):
                while filler and filler[0][2] <= pidx:
                    cost, fn, _ = filler.pop(0)
                    fn()
                    est[0] += cost

            class AttPair:
                """Causal attention for head pair (2*pr, 2*pr+1), query chunk
                qc. Scores for the two heads issue adjacently into different
                PE row groups (partitions 0-63 vs 64-127) and overlap. PV
                runs one kb behind exp so the PE always has exp-independent
                work in flight. Emission is split prologue/body/tail so the
                next pair's first score chunks issue before this pair's tail
                — the scalar engine never waits at a pair boundary."""

                def __init__(self, qc, pr):
                    self.qc, self.pr = qc, pr
                    self.nkb = 4 * qc + 4
                    self.q0 = qc * TC
                    self.psos = [
                        psop.tile([DK + 1, TC], F32, tag="ops", name=f"ops{j}")
                        for j in range(2)
                    ]
                    self.pts = [None] * self.nkb
                    self.chunks = [None] * self.nkb
                    self.emit_scores(0)
                    self.emit_scores(1)

                def emit_scores(self, kb):
                    cs = max(0, kb * 128 - self.q0)
                    # [128, 2, 512] = 2 PSUM banks; pool-rotated (bufs=2) so
                    # the next chunk's scores overlap this chunk's exp
                    ch = ringp.tile([128, 2, TC], F32, tag="spc", name="spc")
                    self.chunks[kb] = (ch, cs)
                    for j, off in enumerate((0, DK)):
                        nc.tensor.matmul(
                            ch[:, j, cs:TC],
                            kT_sb[off : off + DK, self.pr, ts(kb, 128)],
                            qT_sb[off : off + DK, self.pr,
                                  self.q0 + cs : self.q0 + TC],
                            start=True,
                            stop=True,
                        )
                    if kb >= 4 * self.qc:  # diagonal: causal mask
                        for j in range(2):
                            nc.vector.tensor_tensor(
                                ch[:, j, cs : cs + 128],
                                ch[:, j, cs : cs + 128],
                                dm_sb[:],
                                ALU.add,
                            )
                    est[0] += (TC - cs) / 2400 + 0.05

                def emit_exp(self, kb):
                    ch, cs = self.chunks[kb]
                    pt = pt_pool.tile([128, 2, TC], BF16, tag="pt", name="pt")
                    # one strided ACTIVATE covers both heads' slots, trimmed
                    # to the causally-live columns
                    nc.scalar.activation(
                        pt[:, :, cs:TC], ch[:, :, cs:TC], AF.Exp, scale=0.125
                    )
                    self.pts[kb] = pt
                    est[1] += (2 * (TC - cs) + 352) / 1200

                def emit_pv(self, kb):
                    cs = self.chunks[kb][1]
                    pt = self.pts[kb]
                    for j in range(2):
                        nc.tensor.matmul(
                            self.psos[j][:, cs:TC],
                            v_sb[:, kb, 2 * self.pr + j, :],
                            pt[:, j, cs:TC],
                            start=(kb == 0),
                            stop=(kb == self.nkb - 1),
                        )
                    est[0] += 2 * (TC - cs) / 2400 + 0.05

                def body(self):
                    for kb in range(self.nkb):
                        self.emit_exp(kb)
                        if kb > 0:
                            self.emit_pv(kb - 1)
                        if kb + 2 < self.nkb:
                            self.emit_scores(kb + 2)
                        fill()

                def tail(self):
                    self.emit_pv(self.nkb - 1)
                    for j in range(2):
                        ops = self.psos[j]
                        rc = sm_pool.tile([1, TC], F32, tag="rc")
                        # custom-DVE ops mishandle partition-offset inputs:
                        # stage the sums row at partition 0
                        lsb = sm_pool.tile([1, TC], F32, tag="lsb")
                        nc.vector.tensor_copy(lsb[:], ops[DK : DK + 1, :])
                        nc.vector.reciprocal_approx_fast(rc[:], lsb[:])
                        bcs = sm_pool.tile([DK, TC], F32, tag="bcs")
                        nc.gpsimd.partition_broadcast(bcs[:], rc[:])
                        if j == 0:
                            nc.vector.tensor_tensor(
                                aT_sb[0:DK, self.pr, ts(self.qc, TC)],
                                ops[0:DK, :],
                                bcs[:],
                                ALU.mult,
                            )
                        else:
                            tmp = sm_pool.tile([DK, TC], BF16, tag="tmp")
                            nc.vector.tensor_tensor(
                                tmp[:], ops[0:DK, :], bcs[:], ALU.mult
                            )
                            # engines are lane-locked; DMA shifts partitions
                            nc.sync.dma_start(
                                aT_sb[DK:128, self.pr, ts(self.qc, TC)],
                                tmp[:],
                            )

            # ---- emission schedule ----
            # Minimal eager prologue: Q/K for dblk 0 and the four V chains
            # of token chunk 0 — exactly what attention (qc=0, pair 0)
            # consumes — so the scalar engine starts exp() as early as
            # possible. Everything else (remaining QKV chains, the output
            # projection) drips into the attention loop as paced filler so
            # the PE never idles long enough for HAM to re-throttle it to
            # 1.2 GHz. Deadlines are pair indices (qc*4+pr): drain() force-
            # emits a unit before the attention pair that consumes it.
            qkvf = lambda t, u, dl: (1.8, lambda: qkv_chain(t, u), dl)
            oprf = lambda t, h: (1.0, lambda: oproj_half(t, h), 99)

            def qkv_filler(t):
                # V chains + Q/K dblk0 gate pair (t, 0); Q/K dblk p gates
                # pair (t, p) — spread deadlines so drain() never bursts
                # all 12 chains at a qc boundary
                base = 4 * t
                units = [qkvf(t, u, base) for u in (8, 9, 10, 11, 0, 4)]
                units += [qkvf(t, u, base + p) for p in (1, 2, 3)
                          for u in (p, 4 + p)]
                return units

            emit_qkv(0)
            filler += qkv_filler(1)
            prev = None
            for qc in range(NTC):
                if qc == 0:
                    filler += qkv_filler(2)
                elif qc == 1:
                    filler += qkv_filler(3)
                elif qc == 2:
                    filler += [oprf(t, h) for t in range(0, 4)
                               for h in range(2)]
                elif qc == 3:
                    filler += [oprf(t, h) for t in range(4, 12)
                               for h in range(2)]
                for pr in range(4):
                    drain(qc * 4 + pr)
                    st = AttPair(qc, pr)   # emits scores(0),(1)
                    if prev is not None:
                        prev.tail()
                    st.body()
                    prev = st
            # flush leftover filler BEFORE the last pair's tail: the PE
            # chews through it while the final exps/normalize drain
            tail_mode[0] = True
            for _, fn, _d in filler:
                fn()
            prev.tail()
            for tb in range(12, 16):
                for half in range(2):
                    oproj_half(tb, half)

            if debug_dumps:
                nc.sync.dma_start(
                    qT_d.rearrange("(d p) t -> p d t", p=128), qT_sb[:]
                )
                nc.sync.dma_start(
                    kT_d.rearrange("(d p) t -> p d t", p=128), kT_sb[:]
                )
                nc.sync.dma_start(
                    v_d[:], v_sb.rearrange("p a b c -> p (a b c)")
                )
                nc.sync.dma_start(
                    aT_d.rearrange("(d p) t -> p d t", p=128), aT_sb[:]
                )
    nc.compile()
    return nc


_NC = None


def _get_nc():
    global _NC
    if _NC is None:
        _NC = build_nc()
    return _NC


def _numpy_reference(x, attn_mask, wq, bq, wk, bk, wv, bv, wo, bo):
    """Fallback for a non-causal mask (never hit with the standard inputs)."""
    Bsz, Seq, D = x.shape
    scale = 1.0 / np.sqrt(DK)

    def proj(w, b):
        y = x @ w.T + b
        return y.reshape(Bsz, Seq, HEADS, DK).transpose(0, 2, 1, 3)

    q, k, v = proj(wq, bq), proj(wk, bk), proj(wv, bv)
    scores = np.einsum("bhqd,bhkd->bhqk", q, k) * scale
    scores = np.where(attn_mask == 0, np.float32(-1e9), scores)
    scores = scores - scores.max(axis=-1, keepdims=True)
    p = np.exp(scores)
    p /= p.sum(axis=-1, keepdims=True)
    o = np.einsum("bhqk,bhkd->bhqd", p, v)
    o = o.transpose(0, 2, 1, 3).reshape(Bsz, Seq, D)
    return o @ wo.T + bo


def kernel(x, attn_mask, wq, bq, wk, bk, wv, bv, wo, bo, **_unused):
    x = np.asarray(x, np.float32)
    attn_mask = np.asarray(attn_mask)
    wq, bq = np.asarray(wq, np.float32), np.asarray(bq, np.float32)
    wk, bk = np.asarray(wk, np.float32), np.asarray(bk, np.float32)
    wv, bv = np.asarray(wv, np.float32), np.asarray(bv, np.float32)
    wo, bo = np.asarray(wo, np.float32), np.asarray(bo, np.float32)

    causal = np.array_equal(
        np.asarray(attn_mask).reshape(S, S) != 0, np.tril(np.ones((S, S), bool))
    )
    if not causal:
        return _numpy_reference(x, attn_mask, wq, bq, wk, bk, wv, bv, wo, bo)

    def b16(a):
        return np.ascontiguousarray(a, np.float32).astype(ml_dtypes.bfloat16)

    tri = np.where(
        np.arange(128)[:, None] <= np.arange(128)[None, :], 0.0, NEG
    ).astype(np.float32)

    in_maps = []
    for c in range(NCORES):
        b, hg = c // 2, c % 2
        sl = slice(hg * HSL, (hg + 1) * HSL)
        in_maps.append(
            {
                "xT": b16(x[b].T),
                "wqT": b16(wq[sl, :].T),
                "wkT": b16(wk[sl, :].T),
                "wvT": b16(wv[sl, :].T),
                "woT": b16(wo[:, sl].T),
                "bq": np.ascontiguousarray(bq[sl]),
                "bk": np.ascontiguousarray(bk[sl]),
                "bv_rep": np.tile(bv[sl][None, :], (128, 1)),
                "dmask": tri,
            }
        )

    res = run_bass_kernel_spmd(
        _get_nc(), in_maps, core_ids=list(range(NCORES)), **_RUN_KWARGS
    )
    if _RUN_RESULTS is not None:
        _RUN_RESULTS.append(res)

    out = np.empty((B, S, HID), np.float32)
    for b in range(B):
        out[b] = (
            res.results[2 * b]["out_p"].astype(np.float32)
            + res.results[2 * b + 1]["out_p"].astype(np.float32)
            + bo
        )
    return out


# test.py can set these to enable tracing / inspect profile results.
_RUN_KWARGS = {}
_RUN_RESULTS = None
